# revision 1
# baseline (speedup 1.0000x reference)
"""Trainium2 Bass kernel for nn_Attention_23424751632639.

Computation (per (b,h)):  out = tril_strict(rope(Q) @ rope(Q).T / sqrt(N)) @ V
Reformulated as chunked linear attention (exact, just reordered sums):
  out_c = QR_c @ M_c  +  strict_mask(QR_c @ QR_c^T) @ V_c
  M_{c+1} = M_c + QR_c^T @ V_c            (M is the [64,64] running state)
with QR = rope(Q) * N**-0.25 (scale folded into the cos/sin tables, so the
score scale N**-0.5 appears automatically in both the intra and inter terms).

RoPE is computed as  QR = Q*CC + swap(Q)*SS  where swap exchanges feature
pairs (2m <-> 2m+1) and the rotation sign is folded into SS.  swap runs on
GpSimd (otherwise idle), the three wide elementwise ops on DVE.

Matmul operands are bf16 (PE 1 cyc/row vs 4 for fp32); all accumulation
(PSUM, the M state) stays fp32.  QR^T strips come from PE transposes.
Two heads are processed interleaved at chunk-pair granularity so the serial
state -> M-cast -> inter chain of one head hides behind the other head's
matmuls; chunks are paired in PSUM so DVE/ACT fixups run at double width.
The first pipeline stage covers only 2 chunks so the PE starts early.

Sharding: B*H = 32 (b,h) pairs -> 4 per core across 8 cores; no collectives.
"""

import math
import sys

import numpy as np

if "/opt/trn_rl_repo" not in sys.path:
    sys.path.insert(0, "/opt/trn_rl_repo")

B, H, T, N = 2, 16, 4096, 64
THETA = 2.0 ** 16
NCORES = 8
HPC = (B * H) // NCORES  # heads per core


def _host_tables(t_len):
    """Full-width scaled RoPE tables CC, SS [t_len, N] float32."""
    n = np.arange(N, dtype=np.float64)
    tq = np.floor(n / 2.0) * 2.0
    freqs = 1.0 / (THETA ** (tq / N)) / (2.0 * math.pi)  # [N]
    t = np.arange(t_len, dtype=np.float64)[:, None]
    ang = ((t * freqs[None, :]) % 1.0) * (2.0 * math.pi)  # [t_len, N]
    scale = float(N) ** -0.25
    cc = (np.cos(ang) * scale).astype(np.float32)
    ss = (np.sin(ang) * scale).astype(np.float32)
    ss[:, 0::2] *= -1.0
    return np.ascontiguousarray(cc), np.ascontiguousarray(ss)


def _stages(ch):
    """Pipeline stage sizes (chunks): small first stage for a fast start."""
    if ch <= 4:
        return [ch]
    out = [2, 3, 3]
    left = ch - 8
    while left > 0:
        out.append(min(8, left))
        left -= 8
    return out


def build_program(t_len=T, hpc=HPC):
    import concourse.mybir as mybir
    import concourse.tile as tile
    from concourse import bacc

    f32 = mybir.dt.float32
    bf = mybir.dt.bfloat16
    ch = t_len // 128  # number of 128-row chunks per head
    group = min(2, hpc)  # heads interleaved together

    nc = bacc.Bacc(None, target_bir_lowering=False)
    q = nc.dram_tensor("q", [hpc, t_len, N], f32, kind="ExternalInput")
    v = nc.dram_tensor("v", [hpc, t_len, N], f32, kind="ExternalInput")
    cc = nc.dram_tensor("cc", [t_len, N], f32, kind="ExternalInput")
    ss = nc.dram_tensor("ss", [t_len, N], f32, kind="ExternalInput")
    mu = nc.dram_tensor("mu", [128, 256], f32, kind="ExternalInput")
    ident = nc.dram_tensor("ident", [128, 128], bf, kind="ExternalInput")
    o = nc.dram_tensor("o", [hpc, t_len, N], f32, kind="ExternalOutput")

    with tile.TileContext(nc) as tc:
        with (
            tc.tile_pool(name="const", bufs=1) as constp,
            tc.tile_pool(name="head", bufs=2) as headp,
            tc.tile_pool(name="rope", bufs=3) as ropep,
            tc.tile_pool(name="work", bufs=3) as workp,
            tc.tile_pool(name="ps", bufs=2, space="PSUM") as psp,
            tc.tile_pool(name="psm", bufs=1, space="PSUM") as psmp,
        ):
            cc_sb = constp.tile([128, ch * N], f32)
            ss_sb = constp.tile([128, ch * N], f32)
            mu_sb = constp.tile([128, 256], f32)  # [mask | mask] for pairs
            id_sb = constp.tile([128, 128], bf)
            nc.sync.dma_start(
                cc_sb.rearrange("p (c n) -> p c n", c=ch),
                cc.rearrange("(c p) n -> p c n", p=128),
            )
            nc.sync.dma_start(
                ss_sb.rearrange("p (c n) -> p c n", c=ch),
                ss.rearrange("(c p) n -> p c n", p=128),
            )
            nc.sync.dma_start(mu_sb[:], mu[:])
            nc.sync.dma_start(id_sb[:], ident[:])

            for hg in range(hpc // group):
                heads = [hg * group + i for i in range(group)]
                qr = {}
                qrt = {}
                v_sb = {}
                m_ps = {}
                mb_prev = {}
                ost = {}

                # load + rope + transposes, pipelined in stages
                cbase = 0
                for stage in _stages(ch):
                    csl = slice(cbase, cbase + stage)
                    fsl = slice(cbase * N, (cbase + stage) * N)
                    for h in heads:
                        if cbase == 0:
                            qr[h] = headp.tile(
                                [128, ch * N], bf, name=f"qr{h}",
                                tag=f"qr{h % group}")
                            qrt[h] = headp.tile(
                                [64, ch * 128], bf, name=f"qrt{h}",
                                tag=f"qrt{h % group}")
                            v_sb[h] = headp.tile(
                                [128, ch * N], bf, name=f"v{h}",
                                tag=f"v{h % group}")
                        q_sb = ropep.tile([128, 8 * N], f32, name="qst", tag="q")[
                            :, :stage * N]
                        v32 = ropep.tile([128, 8 * N], f32, name="v32", tag="v32")[
                            :, :stage * N]
                        nc.sync.dma_start(
                            q_sb.rearrange("p (c n) -> p c n", c=stage),
                            q[h].rearrange("(c p) n -> p c n", p=128)[:, csl],
                        )
                        nc.sync.dma_start(
                            v32.rearrange("p (c n) -> p c n", c=stage),
                            v[h].rearrange("(c p) n -> p c n", p=128)[:, csl],
                        )
                        nc.vector.tensor_copy(v_sb[h][:, fsl], v32)

                        # swap(Q): exchange feature pairs -> bf16 (GpSimd)
                        swp = ropep.tile([128, 8 * N], bf, name="swp", tag="swp")[
                            :, :stage * N]
                        sw4 = swp.rearrange("p (c m o) -> p c m o",
                                            c=stage, m=32, o=2)
                        q4 = q_sb.rearrange("p (c m o) -> p c m o",
                                            c=stage, m=32, o=2)
                        nc.gpsimd.tensor_copy(sw4[:, :, :, 0], q4[:, :, :, 1])
                        nc.gpsimd.tensor_copy(sw4[:, :, :, 1], q4[:, :, :, 0])

                        # QR = Q*CC + swap(Q)*SS   (contiguous DVE ops)
                        t1 = ropep.tile([128, 8 * N], f32, name="t1", tag="t1")[
                            :, :stage * N]
                        t2 = ropep.tile([128, 8 * N], f32, name="t2", tag="t2")[
                            :, :stage * N]
                        nc.vector.tensor_mul(t1, q_sb, cc_sb[:, fsl])
                        nc.vector.tensor_mul(t2, swp, ss_sb[:, fsl])
                        nc.vector.tensor_add(qr[h][:, fsl], t1, t2)

                        # QR^T strips via PE transpose, one ACT copy per pair
                        for cp in range(cbase // 2, (cbase + stage) // 2):
                            c0, c1 = 2 * cp, 2 * cp + 1
                            tr_ps = psp.tile([64, 256], bf, tag="tr")
                            nc.tensor.transpose(
                                tr_ps[:, 0:128],
                                qr[h][:, c0 * 64:(c0 + 1) * 64], id_sb[:],
                            )
                            nc.tensor.transpose(
                                tr_ps[:, 128:256],
                                qr[h][:, c1 * 64:(c1 + 1) * 64], id_sb[:],
                            )
                            nc.scalar.copy(
                                qrt[h][:, cp * 256:(cp + 1) * 256], tr_ps[:]
                            )
                    cbase += stage

                for h in heads:
                    m_ps[h] = psmp.tile([128, 64], f32, name=f"m{h}",
                                        tag=f"m{h % group}", bufs=1)

                for cp in range(ch // 2):
                    c0, c1 = 2 * cp, 2 * cp + 1
                    for h in heads:
                        # intra: P blocks for both chunks into one PSUM tile,
                        # one masked copy (strict-upper mask doubled)
                        p_ps = psp.tile([128, 256], f32, tag="p")
                        p_sb = workp.tile([128, 256], bf, tag="psb")
                        out_ps = psp.tile([128, 128], f32, tag="out")
                        for k, c in ((0, c0), (1, c1)):
                            qrt_c = qrt[h][:, c * 128:(c + 1) * 128]
                            nc.tensor.matmul(
                                p_ps[:, k * 128:(k + 1) * 128], qrt_c, qrt_c,
                                start=True, stop=True,
                            )
                        nc.vector.tensor_mul(p_sb[:], p_ps[:], mu_sb[:])

                        for k, c in ((0, c0), (1, c1)):
                            qrt_c = qrt[h][:, c * 128:(c + 1) * 128]
                            v_c = v_sb[h][:, c * 64:(c + 1) * 64]
                            qr_c = qr[h][:, c * 64:(c + 1) * 64]
                            osl = slice(k * 64, (k + 1) * 64)
                            if c == 0:
                                nc.tensor.matmul(
                                    out_ps[:, osl],
                                    p_sb[:, k * 128:(k + 1) * 128],
                                    v_c, start=True, stop=True,
                                )
                            else:
                                # inter: out += QR_c @ M (state after c-1)
                                nc.tensor.matmul(
                                    out_ps[:, osl], qrt_c,
                                    mb_prev[h][0:64, :],
                                    start=True, stop=False,
                                )
                                nc.tensor.matmul(
                                    out_ps[:, osl],
                                    p_sb[:, k * 128:(k + 1) * 128],
                                    v_c, start=False, stop=True,
                                )

                            # state: M += QR_c^T @ V_c, accumulated in PSUM
                            nc.tensor.matmul(
                                m_ps[h][0:64, :], qr_c, v_c,
                                start=(c == 0), stop=(c == ch - 1),
                                skip_group_check=True,
                            )
                            if c < ch - 1:
                                m_bf = workp.tile([128, 64], bf,
                                                  tag=f"mbf{h % group}")
                                nc.scalar.copy(m_bf[0:64, :], m_ps[h][0:64, :])
                                mb_prev[h] = m_bf

                        # batch output: stage 4 chunks (2 pairs), one DMA
                        k2 = cp % 2
                        if k2 == 0:
                            ost[h] = workp.tile([128, 256], f32,
                                                name=f"ost{h}",
                                                tag=f"ost{h % group}")
                        nc.scalar.copy(
                            ost[h][:, k2 * 128:(k2 + 1) * 128], out_ps[:]
                        )
                        if k2 == 1:
                            g4 = cp // 2 * 4
                            nc.sync.dma_start(
                                o[h].rearrange("(g p) n -> p g n", p=128)[
                                    :, g4:g4 + 4],
                                ost[h].rearrange("p (g n) -> p g n", g=4),
                            )

    nc.compile()
    return nc


_CACHE = {}


def _get_program():
    if "nc" not in _CACHE:
        _CACHE["nc"] = build_program()
    return _CACHE["nc"]


def _strict_upper_mask():
    # lhsT for the diag block: keep P[j, i] where j < i; doubled for pairs
    m = np.triu(np.ones((128, 128), dtype=np.float32), k=1)
    return np.ascontiguousarray(np.concatenate([m, m], axis=1))


def _identity():
    import ml_dtypes

    return np.eye(128, dtype=ml_dtypes.bfloat16)


def kernel(Q, V):
    from concourse.bass_utils import run_bass_kernel_spmd

    Q = np.ascontiguousarray(np.asarray(Q), dtype=np.float32)
    V = np.ascontiguousarray(np.asarray(V), dtype=np.float32)
    qf = Q.reshape(NCORES, HPC, T, N)
    vf = V.reshape(NCORES, HPC, T, N)
    cc, ss = _host_tables(T)
    mu = _strict_upper_mask()
    ident = _identity()

    nc = _get_program()
    in_maps = [
        {"q": qf[i], "v": vf[i], "cc": cc, "ss": ss, "mu": mu, "ident": ident}
        for i in range(NCORES)
    ]
    res = run_bass_kernel_spmd(nc, in_maps, core_ids=list(range(NCORES)))
    out = np.stack([r["o"] for r in res.results], axis=0)
    return out.reshape(B, H, T, N)



# revision 11
# speedup vs baseline: 1.9681x; 1.9681x over previous
"""Trainium2 Bass kernel for nn_Attention_23424751632639.

Computation (per (b,h)):  out = tril_strict(rope(Q) @ rope(Q).T / sqrt(N)) @ V
Chunked linear attention (exact reordering of the sums), chunk = 128 rows:
  out_c = QR_c @ M_{c-1}  +  strict_mask(QR_c @ QR_c^T) @ V_c
  M_c   = M_{c-1} + QR_c^T @ V_c          (M = running [64,64] state, PSUM)

Implementation (v3):
  * fp16 everywhere on device; all matmul accumulation stays fp32 in PSUM.
  * RoPE (elementwise) is applied on the host; the device receives QR in both
    natural [t, n] and transposed [n, t] layouts plus V, all fp16, pre-laid
    out per-partition so every DMA moves multi-KB contiguous runs (13 total
    dma_starts).  The scores scale N**-0.5 is folded into the rope tables.
  * Per chunk (4 heads) the PE runs: 4 state matmuls, 4 S blocks + 4 inter
    matmuls (S and inter share the same qrt stationary operand), 4 intra
    matmuls.  All matmul operands sit at partition base 0 (base-64 operands
    fault the device).
  * intra(c) is issued one chunk late so the strict-mask multiply (on
    DVE/ACT/GpSimd) never stalls the PE.
  * PSUM zero-region discipline: one start=True on the first write of each
    2KB region, one stop=True on the last; everything between accumulates.
  * PSUM->SBUF crossings (P-mask, M snapshot, output copy) are statically
    rotated across DVE / ACT / GpSimd.

Sharding: B*H = 32 (b,h) pairs -> 4 per core across 8 cores; no collectives.
"""

import math
import sys

import numpy as np

if "/opt/trn_rl_repo" not in sys.path:
    sys.path.insert(0, "/opt/trn_rl_repo")

B, H, T, N = 2, 16, 4096, 64
THETA = 2.0 ** 16
NCORES = 8
HPC = (B * H) // NCORES   # heads per core
CH = T // 128             # chunks per head (32)
NW = 4                    # windows
CPW = CH // NW            # chunks per window (8)
WCOLS = CPW * HPC * N     # columns per (window, stream) slice (2048)


def build_program():
    import concourse.mybir as mybir
    import concourse.tile as tile
    from concourse import bacc

    f32 = mybir.dt.float32
    f16 = mybir.dt.float16

    nc = bacc.Bacc(None, target_bir_lowering=False)
    # qn: [p, w, s, h, cw, n]; s: 0=qr 1=v       (natural layouts)
    qn = nc.dram_tensor("qn", [128, NW * 2 * WCOLS], f16, kind="ExternalInput")
    # qt: [p(n), w, h, cw, t]                    (transposed rope(Q))
    qt = nc.dram_tensor("qt", [64, NW * 2 * WCOLS], f16, kind="ExternalInput")
    cst = nc.dram_tensor("cst", [128, 512], f16, kind="ExternalInput")
    # o: [p, w, cw, h, n]
    o = nc.dram_tensor("o", [128, NW * WCOLS], f16, kind="ExternalOutput")

    with tile.TileContext(nc) as tc:
        with (
            tc.tile_pool(name="big", bufs=1) as bigp,
            tc.tile_pool(name="mb", bufs=2) as mbp,
            tc.tile_pool(name="psb", bufs=3) as psbp,
            tc.tile_pool(name="tmp", bufs=2) as tmpp,
            tc.tile_pool(name="ost", bufs=2) as ostp,
            tc.tile_pool(name="spps", bufs=2, space="PSUM") as spp,
            tc.tile_pool(name="outps", bufs=4, space="PSUM") as outp,
            tc.tile_pool(name="mps", bufs=1, space="PSUM") as mpp,
        ):
            qn_sb = bigp.tile([128, NW * 2 * WCOLS], f16)
            qt_sb = bigp.tile([64, NW * 2 * WCOLS], f16)
            cst_sb = bigp.tile([128, 512], f16)
            mask4 = cst_sb[:, 0:512]

            qn4 = qn.rearrange("p (w s q) -> p w s q", w=NW, s=2)
            qnsb4 = qn_sb.rearrange("p (w s q) -> p w s q", w=NW, s=2)
            qt4 = qt.rearrange("p (w q) -> p w q", w=NW)
            qtsb4 = qt_sb.rearrange("p (w q) -> p w q", w=NW)

            def dma_qn(w, qlo, qhi):
                nc.sync.dma_start(qnsb4[:, w, :, qlo:qhi], qn4[:, w, :, qlo:qhi])

            def dma_qt(w, qlo, qhi):
                nc.sync.dma_start(qtsb4[:, w, qlo:qhi], qt4[:, w, qlo:qhi])

            mreg = mpp.tile([64, 256], f32, name="mreg")

            # lagged-intra bookkeeping: body(c) consumes chunk c-1's tiles
            prev = {}

            def body(c):
                w, cl = c // CPW, c % CPW
                pw, k = cl // 2, cl % 2
                base = w * 2 * WCOLS

                def qr_sl(h):  # [128, 64] natural rope(Q) chunk
                    off = base + 512 * h + 64 * cl
                    return qn_sb[:, off:off + 64]

                def v_sl(h):   # [128, 64] V chunk
                    off = base + WCOLS + 512 * h + 64 * cl
                    return qn_sb[:, off:off + 64]

                def qrt_sl(h):  # [64, 128] transposed rope(Q) chunk
                    off = 2 * w * WCOLS + 1024 * h + 128 * cl
                    return qt_sb[:, off:off + 128]

                # state: M_h += QR_c^T V_c   (PSUM accumulate across chunks)
                for h in range(HPC):
                    nc.tensor.matmul(
                        mreg[:, 64 * h:64 * h + 64],
                        qr_sl(h), v_sl(h),
                        start=(c == 0 and h == 0),
                        stop=(c == CH - 1 and h == HPC - 1),
                        skip_group_check=True,
                    )

                # M snapshot for inter of chunk c+1
                mb = None
                if c < CH - 1:
                    mb = mbp.tile([64, 256], f16, tag="mb")
                    if c % 2 == 0:
                        nc.vector.tensor_copy(mb[:], mreg[:])
                    else:
                        nc.scalar.copy(mb[:], mreg[:])

                # output PSUM tile per pair
                if k == 0:
                    op = outp.tile([128, 512], f32, tag="outp")
                    prev["outp"] = op
                else:
                    op = prev["outp"]

                # S blocks (+ inter sharing the same stationary operand)
                sp = spp.tile([128, 512], f32, tag="sp")
                mb_prev = prev.get("mb")
                for h in range(HPC):
                    qrt_c = qrt_sl(h)
                    nc.tensor.matmul(
                        sp[:, 128 * h:128 * h + 128], qrt_c, qrt_c,
                        start=(h == 0), stop=(h == HPC - 1),
                    )
                    if c > 0:
                        # first write of this pair's outp zero region gets
                        # start=True (inter of even chunk; chunk 1 for pair 0)
                        nc.tensor.matmul(
                            op[:, 256 * k + 64 * h:256 * k + 64 * h + 64],
                            qrt_c, mb_prev[:, 64 * h:64 * h + 64],
                            start=(h == 0 and (k == 0 or c == 1)),
                            stop=False,
                        )

                # P = S * strict-upper mask  (psum f32 -> sbuf fp16)
                psb = psbp.tile([128, 512], f16, tag="psb")
                r = c % 4
                if r in (0, 2):
                    nc.vector.tensor_mul(psb[:], sp[:], mask4)
                else:
                    tmp = tmpp.tile([128, 512], f16, tag="tmp")
                    nc.scalar.copy(tmp[:], sp[:])
                    if r == 1:
                        nc.vector.tensor_mul(psb[:], tmp[:], mask4)
                    else:
                        nc.gpsimd.tensor_mul(psb[:], tmp[:], mask4)

                # lagged intra of chunk c-1 (+ output copy / DMA per pair)
                if c > 0:
                    intra(c - 1)

                prev["mb"] = mb
                prev["psb_c"] = psb
                prev["v_c"] = [v_sl(h) for h in range(HPC)]
                prev["op_c"] = op

            ost_t = [None] * NW

            def intra(c):
                w, cl = c // CPW, c % CPW
                pw, k = cl // 2, cl % 2
                psb = prev["psb_c"]
                op = prev["op_c"]
                vsl = prev["v_c"]
                for h in range(HPC):
                    nc.tensor.matmul(
                        op[:, 256 * k + 64 * h:256 * k + 64 * h + 64],
                        psb[:, 128 * h:128 * h + 128], vsl[h],
                        start=False, stop=(k == 1 and h == HPC - 1),
                    )
                if k == 1:
                    # pair finished: copy to fp16 staging, DMA per window
                    if ost_t[w] is None:
                        ost_t[w] = ostp.tile([128, WCOLS], f16,
                                             name=f"ost{w}", tag="ost")
                    dst = ost_t[w][:, 512 * pw:512 * pw + 512]
                    if pw % 4 in (0, 3):
                        nc.vector.tensor_copy(dst, op[:])
                    else:
                        nc.scalar.copy(dst, op[:])
                    if pw == CPW // 2 - 1:
                        nc.sync.dma_start(
                            o[:, w * WCOLS:(w + 1) * WCOLS], ost_t[w][:]
                        )
                        ost_t[w] = None

            # ---- schedule ----
            nc.sync.dma_start(cst_sb[:], cst[:])
            dma_qn(0, 0, WCOLS)
            dma_qt(0, 0, 2 * WCOLS)

            for c in range(CH):
                w, cl = c // CPW, c % CPW
                if w < NW - 1:
                    if cl == 1:
                        dma_qn(w + 1, 0, WCOLS)
                    elif cl == 2:
                        dma_qt(w + 1, 0, 2 * WCOLS)
                body(c)
            intra(CH - 1)

    nc.compile()
    return nc


_CACHE = {}


def _get_program():
    if "nc" not in _CACHE:
        _CACHE["nc"] = build_program()
    return _CACHE["nc"]


def _tables():
    n = np.arange(N, dtype=np.float64)
    tq = np.floor(n / 2.0) * 2.0
    freqs = 1.0 / (THETA ** (tq / N)) / (2.0 * math.pi)
    t = np.arange(T, dtype=np.float64)[:, None]
    ang = ((t * freqs[None, :]) % 1.0) * (2.0 * math.pi)
    scale = float(N) ** -0.25
    cc = (np.cos(ang) * scale).astype(np.float32)
    ss = (np.sin(ang) * scale).astype(np.float32)
    ss[:, 0::2] *= -1.0
    return cc, ss


def make_inputs(Q, V):
    """Full inputs -> list of per-core {'qn','qt','cst'} fp16 host arrays."""
    Q = np.asarray(Q, dtype=np.float32).reshape(NCORES, HPC, T, N)
    V = np.asarray(V, dtype=np.float32).reshape(NCORES, HPC, T, N)
    cc, ss = _tables()
    sq = np.empty_like(Q)
    sq[..., 0::2] = Q[..., 1::2]
    sq[..., 1::2] = Q[..., 0::2]
    qr = (Q * cc + sq * ss).astype(np.float16)  # scaled rope(Q)
    v16 = V.astype(np.float16)

    # natural: [core, h, w, cw, p, n] -> [core, p, w, (h cw n)]
    def nat(x):
        x = x.reshape(NCORES, HPC, NW, CPW, 128, N)
        return np.transpose(x, (0, 4, 2, 1, 3, 5))  # core p w h cw n

    qn_h = np.stack([nat(qr), nat(v16)], axis=3)  # core p w s h cw n
    qn_h = np.ascontiguousarray(qn_h.reshape(NCORES, 128, NW * 2 * WCOLS))

    # transposed: [core, n, w, (h cw t)]
    qt_h = qr.reshape(NCORES, HPC, NW, CPW, 128, N)
    qt_h = np.transpose(qt_h, (0, 5, 2, 1, 3, 4))  # core n w h cw t
    qt_h = np.ascontiguousarray(qt_h.reshape(NCORES, 64, NW * 2 * WCOLS))

    mu = np.triu(np.ones((128, 128), dtype=np.float16), k=1)
    cst = np.ascontiguousarray(np.concatenate([mu] * 4, axis=1))  # [128, 512]
    return [{"qn": qn_h[i], "qt": qt_h[i], "cst": cst}
            for i in range(NCORES)]


def unpack_out(results):
    """list of per-core {'o': [128, NW*WCOLS] fp16} -> [B,H,T,N] f32."""
    o = np.stack([r["o"] for r in results], axis=0)
    o = o.reshape(NCORES, 128, NW, CPW, HPC, N)
    o = np.transpose(o, (0, 4, 2, 3, 1, 5))  # [8, HPC, NW, CPW, 128, N]
    return np.ascontiguousarray(
        o.reshape(B, H, T, N).astype(np.float32))


def kernel(Q, V):
    from concourse.bass_utils import run_bass_kernel_spmd

    nc = _get_program()
    in_maps = make_inputs(Q, V)
    res = run_bass_kernel_spmd(nc, in_maps, core_ids=list(range(NCORES)))
    return unpack_out(res.results)


# revision 14
# speedup vs baseline: 2.0630x; 1.0482x over previous
"""Trainium2 Bass kernel for nn_Attention_23424751632639.

Computation (per (b,h)):  out = tril_strict(rope(Q) @ rope(Q).T / sqrt(N)) @ V
Chunked linear attention (exact reordering of the sums), chunk = 128 rows:
  out_c = QR_c @ M_{c-1}  +  strict_mask(QR_c @ QR_c^T) @ V_c
  M_c   = M_{c-1} + QR_c^T @ V_c          (M = running [64,64] state, PSUM)

Implementation (v3):
  * fp16 everywhere on device; all matmul accumulation stays fp32 in PSUM.
  * RoPE (elementwise) is applied on the host; the device receives QR in both
    natural [t, n] and transposed [n, t] layouts plus V, all fp16, pre-laid
    out per-partition so every DMA moves multi-KB contiguous runs (13 total
    dma_starts).  The scores scale N**-0.5 is folded into the rope tables.
  * Per chunk (4 heads) the PE runs: 4 state matmuls, 4 S blocks + 4 inter
    matmuls (S and inter share the same qrt stationary operand), 4 intra
    matmuls.  All matmul operands sit at partition base 0 (base-64 operands
    fault the device).
  * intra(c) is issued one chunk late so the strict-mask multiply (on
    DVE/ACT/GpSimd) never stalls the PE.
  * PSUM zero-region discipline: one start=True on the first write of each
    2KB region, one stop=True on the last; everything between accumulates.
  * PSUM->SBUF crossings (P-mask, M snapshot, output copy) are statically
    rotated across DVE / ACT / GpSimd.

Sharding: B*H = 32 (b,h) pairs -> 4 per core across 8 cores; no collectives.
"""

import math
import sys

import numpy as np

if "/opt/trn_rl_repo" not in sys.path:
    sys.path.insert(0, "/opt/trn_rl_repo")

B, H, T, N = 2, 16, 4096, 64
THETA = 2.0 ** 16
NCORES = 8
HPC = (B * H) // NCORES   # heads per core
CH = T // 128             # chunks per head (32)
NW = 4                    # windows
CPW = CH // NW            # chunks per window (8)
WCOLS = CPW * HPC * N     # columns per (window, stream) slice (2048)


def build_program():
    import concourse.mybir as mybir
    import concourse.tile as tile
    from concourse import bacc

    f32 = mybir.dt.float32
    f16 = mybir.dt.float16

    nc = bacc.Bacc(None, target_bir_lowering=False)
    # qn: [p, w, s, cw, h, n]; s: 0=qr 1=v       (natural layouts)
    qn = nc.dram_tensor("qn", [128, NW * 2 * WCOLS], f16, kind="ExternalInput")
    # qt: [p(n), w, cw, h, t]                    (transposed rope(Q))
    qt = nc.dram_tensor("qt", [64, NW * 2 * WCOLS], f16, kind="ExternalInput")
    cst = nc.dram_tensor("cst", [128, 512], f16, kind="ExternalInput")
    # o: [p, w, cw, h, n]
    o = nc.dram_tensor("o", [128, NW * WCOLS], f16, kind="ExternalOutput")

    with tile.TileContext(nc) as tc:
        with (
            tc.tile_pool(name="big", bufs=1) as bigp,
            tc.tile_pool(name="mb", bufs=2) as mbp,
            tc.tile_pool(name="psb", bufs=4) as psbp,
            tc.tile_pool(name="tmp", bufs=3) as tmpp,
            tc.tile_pool(name="ost", bufs=3) as ostp,
            tc.tile_pool(name="spps", bufs=3, space="PSUM") as spp,
            tc.tile_pool(name="outps", bufs=3, space="PSUM") as outp,
            tc.tile_pool(name="mps", bufs=1, space="PSUM") as mpp,
        ):
            qn_sb = bigp.tile([128, NW * 2 * WCOLS], f16)
            qt_sb = bigp.tile([64, NW * 2 * WCOLS], f16)
            cst_sb = bigp.tile([128, 512], f16)
            mask4 = cst_sb[:, 0:512]

            qn4 = qn.rearrange("p (w s q) -> p w s q", w=NW, s=2)
            qnsb4 = qn_sb.rearrange("p (w s q) -> p w s q", w=NW, s=2)
            qt4 = qt.rearrange("p (w q) -> p w q", w=NW)
            qtsb4 = qt_sb.rearrange("p (w q) -> p w q", w=NW)

            def dma_qn(w, clo, chi):
                a, b = 256 * clo, 256 * chi
                nc.sync.dma_start(qnsb4[:, w, :, a:b], qn4[:, w, :, a:b])

            def dma_qt(w, clo, chi):
                a, b = 512 * clo, 512 * chi
                nc.sync.dma_start(qtsb4[:, w, a:b], qt4[:, w, a:b])

            mreg = mpp.tile([64, 256], f32, name="mreg")

            # per-chunk records for the 2-chunk-lagged intra
            rec = {}

            def body(c):
                w, cl = c // CPW, c % CPW
                pw, k = cl // 2, cl % 2
                base = w * 2 * WCOLS

                def qr_sl(h):  # [128, 64] natural rope(Q) chunk
                    off = base + 256 * cl + 64 * h
                    return qn_sb[:, off:off + 64]

                def v_sl(h):   # [128, 64] V chunk
                    off = base + WCOLS + 256 * cl + 64 * h
                    return qn_sb[:, off:off + 64]

                def qrt_sl(h):  # [64, 128] transposed rope(Q) chunk
                    off = 2 * w * WCOLS + 512 * cl + 128 * h
                    return qt_sb[:, off:off + 128]

                # state: M_h += QR_c^T V_c   (PSUM accumulate across chunks)
                for h in range(HPC):
                    nc.tensor.matmul(
                        mreg[:, 64 * h:64 * h + 64],
                        qr_sl(h), v_sl(h),
                        start=(c == 0 and h == 0),
                        stop=(c == CH - 1 and h == HPC - 1),
                        skip_group_check=True,
                    )

                # M snapshot for inter of chunk c+1
                mb = None
                if c < CH - 1:
                    mb = mbp.tile([64, 256], f16, tag="mb")
                    if c % 2 == 0:
                        nc.vector.tensor_copy(mb[:], mreg[:])
                    else:
                        nc.scalar.copy(mb[:], mreg[:])

                # output PSUM tile per pair
                if k == 0:
                    op = outp.tile([128, 512], f32, tag="outp")
                else:
                    op = rec[c - 1]["op"]

                # S blocks (+ inter sharing the same stationary operand)
                sp = spp.tile([128, 512], f32, tag="sp")
                for h in range(HPC):
                    qrt_c = qrt_sl(h)
                    nc.tensor.matmul(
                        sp[:, 128 * h:128 * h + 128], qrt_c, qrt_c,
                        start=(h == 0), stop=(h == HPC - 1),
                    )
                    if c > 0:
                        # first write of this pair's outp zero region gets
                        # start=True (inter of even chunk; chunk 1 for pair 0)
                        nc.tensor.matmul(
                            op[:, 256 * k + 64 * h:256 * k + 64 * h + 64],
                            qrt_c, rec[c - 1]["mb"][:, 64 * h:64 * h + 64],
                            start=(h == 0 and (k == 0 or c == 1)),
                            stop=False,
                        )

                # P = S * strict-upper mask  (psum f32 -> sbuf fp16)
                psb = psbp.tile([128, 512], f16, tag="psb")
                r = c % 4
                if r in (1, 3):
                    nc.vector.tensor_mul(psb[:], sp[:], mask4)
                else:
                    tmp = tmpp.tile([128, 512], f16, tag="tmp")
                    nc.scalar.copy(tmp[:], sp[:])
                    if r == 0:
                        nc.vector.tensor_mul(psb[:], tmp[:], mask4)
                    else:
                        nc.gpsimd.tensor_mul(psb[:], tmp[:], mask4)

                # intra lagged by 2 chunks so the mask never stalls the PE
                if c > 1:
                    intra(c - 2)

                rec[c] = {"mb": mb, "psb": psb, "op": op,
                          "v": [v_sl(h) for h in range(HPC)]}
                rec.pop(c - 3, None)

            def intra(c):
                w, cl = c // CPW, c % CPW
                pw, k = cl // 2, cl % 2
                r = rec[c]
                for h in range(HPC):
                    nc.tensor.matmul(
                        r["op"][:, 256 * k + 64 * h:256 * k + 64 * h + 64],
                        r["psb"][:, 128 * h:128 * h + 128], r["v"][h],
                        start=False, stop=(k == 1 and h == HPC - 1),
                    )
                if k == 1:
                    # pair finished: fp16 staging copy + per-pair output DMA
                    ost = ostp.tile([128, 512], f16, tag="ost")
                    if pw % 2 == 0:
                        nc.scalar.copy(ost[:], r["op"][:])
                    else:
                        nc.vector.tensor_copy(ost[:], r["op"][:])
                    off = w * WCOLS + 512 * pw
                    nc.sync.dma_start(o[:, off:off + 512], ost[:])

            # ---- schedule ----
            nc.sync.dma_start(cst_sb[:], cst[:])
            dma_qn(0, 0, 2)
            dma_qt(0, 0, 2)
            dma_qn(0, 2, CPW)
            dma_qt(0, 2, CPW)

            for c in range(CH):
                w, cl = c // CPW, c % CPW
                if w < NW - 1:
                    if cl == 0:
                        dma_qn(w + 1, 0, CPW)
                    elif cl == 1:
                        dma_qt(w + 1, 0, CPW)
                body(c)
            intra(CH - 2)
            intra(CH - 1)

    nc.compile()
    return nc


_CACHE = {}


def _get_program():
    if "nc" not in _CACHE:
        _CACHE["nc"] = build_program()
    return _CACHE["nc"]


def _tables():
    n = np.arange(N, dtype=np.float64)
    tq = np.floor(n / 2.0) * 2.0
    freqs = 1.0 / (THETA ** (tq / N)) / (2.0 * math.pi)
    t = np.arange(T, dtype=np.float64)[:, None]
    ang = ((t * freqs[None, :]) % 1.0) * (2.0 * math.pi)
    scale = float(N) ** -0.25
    cc = (np.cos(ang) * scale).astype(np.float32)
    ss = (np.sin(ang) * scale).astype(np.float32)
    ss[:, 0::2] *= -1.0
    return cc, ss


def make_inputs(Q, V):
    """Full inputs -> list of per-core {'qn','qt','cst'} fp16 host arrays."""
    Q = np.asarray(Q, dtype=np.float32).reshape(NCORES, HPC, T, N)
    V = np.asarray(V, dtype=np.float32).reshape(NCORES, HPC, T, N)
    cc, ss = _tables()
    sq = np.empty_like(Q)
    sq[..., 0::2] = Q[..., 1::2]
    sq[..., 1::2] = Q[..., 0::2]
    qr = (Q * cc + sq * ss).astype(np.float16)  # scaled rope(Q)
    v16 = V.astype(np.float16)

    # natural: [core, h, w, cw, p, n] -> [core, p, w, (cw h n)]
    def nat(x):
        x = x.reshape(NCORES, HPC, NW, CPW, 128, N)
        return np.transpose(x, (0, 4, 2, 3, 1, 5))  # core p w cw h n

    qn_h = np.stack([nat(qr), nat(v16)], axis=3)  # core p w s cw h n
    qn_h = np.ascontiguousarray(qn_h.reshape(NCORES, 128, NW * 2 * WCOLS))

    # transposed: [core, n, w, (cw h t)]
    qt_h = qr.reshape(NCORES, HPC, NW, CPW, 128, N)
    qt_h = np.transpose(qt_h, (0, 5, 2, 3, 1, 4))  # core n w cw h t
    qt_h = np.ascontiguousarray(qt_h.reshape(NCORES, 64, NW * 2 * WCOLS))

    mu = np.triu(np.ones((128, 128), dtype=np.float16), k=1)
    cst = np.ascontiguousarray(np.concatenate([mu] * 4, axis=1))  # [128, 512]
    return [{"qn": qn_h[i], "qt": qt_h[i], "cst": cst}
            for i in range(NCORES)]


def unpack_out(results):
    """list of per-core {'o': [128, NW*WCOLS] fp16} -> [B,H,T,N] f32."""
    o = np.stack([r["o"] for r in results], axis=0)
    o = o.reshape(NCORES, 128, NW, CPW, HPC, N)
    o = np.transpose(o, (0, 4, 2, 3, 1, 5))  # [8, HPC, NW, CPW, 128, N]
    return np.ascontiguousarray(
        o.reshape(B, H, T, N).astype(np.float32))


def kernel(Q, V):
    from concourse.bass_utils import run_bass_kernel_spmd

    nc = _get_program()
    in_maps = make_inputs(Q, V)
    res = run_bass_kernel_spmd(nc, in_maps, core_ids=list(range(NCORES)))
    return unpack_out(res.results)


# revision 16
# speedup vs baseline: 2.0680x; 1.0024x over previous
"""Trainium2 Bass kernel for nn_Attention_23424751632639.

Computation (per (b,h)):  out = tril_strict(rope(Q) @ rope(Q).T / sqrt(N)) @ V
Chunked linear attention (exact reordering of the sums), chunk = 128 rows:
  out_c = QR_c @ M_{c-1}  +  strict_mask(QR_c @ QR_c^T) @ V_c
  M_c   = M_{c-1} + QR_c^T @ V_c          (M = running [64,64] state, PSUM)

Implementation (v3):
  * fp16 everywhere on device; all matmul accumulation stays fp32 in PSUM.
  * RoPE (elementwise) is applied on the host; the device receives QR in both
    natural [t, n] and transposed [n, t] layouts plus V, all fp16, pre-laid
    out per-partition so every DMA moves multi-KB contiguous runs (13 total
    dma_starts).  The scores scale N**-0.5 is folded into the rope tables.
  * Per chunk (4 heads) the PE runs: 4 state matmuls, 4 S blocks + 4 inter
    matmuls (S and inter share the same qrt stationary operand), 4 intra
    matmuls.  All matmul operands sit at partition base 0 (base-64 operands
    fault the device).
  * intra(c) is issued one chunk late so the strict-mask multiply (on
    DVE/ACT/GpSimd) never stalls the PE.
  * PSUM zero-region discipline: one start=True on the first write of each
    2KB region, one stop=True on the last; everything between accumulates.
  * PSUM->SBUF crossings (P-mask, M snapshot, output copy) are statically
    rotated across DVE / ACT / GpSimd.

Sharding: B*H = 32 (b,h) pairs -> 4 per core across 8 cores; no collectives.
"""

import math
import sys

import numpy as np

if "/opt/trn_rl_repo" not in sys.path:
    sys.path.insert(0, "/opt/trn_rl_repo")

B, H, T, N = 2, 16, 4096, 64
THETA = 2.0 ** 16
NCORES = 8
HPC = (B * H) // NCORES   # heads per core
CH = T // 128             # chunks per head (32)
NW = 4                    # windows
CPW = CH // NW            # chunks per window (8)
WCOLS = CPW * HPC * N     # columns per (window, stream) slice (2048)


def build_program():
    import concourse.mybir as mybir
    import concourse.tile as tile
    from concourse import bacc

    f32 = mybir.dt.float32
    f16 = mybir.dt.float16

    nc = bacc.Bacc(None, target_bir_lowering=False)
    # qn: [p, w, s, cw, h, n]; s: 0=qr 1=v       (natural layouts)
    qn = nc.dram_tensor("qn", [128, NW * 2 * WCOLS], f16, kind="ExternalInput")
    # qt: [p(n), w, cw, h, t]                    (transposed rope(Q))
    qt = nc.dram_tensor("qt", [64, NW * 2 * WCOLS], f16, kind="ExternalInput")
    cst = nc.dram_tensor("cst", [128, 512], f16, kind="ExternalInput")
    # o: [p, w, cw, h, n]
    o = nc.dram_tensor("o", [128, NW * WCOLS], f16, kind="ExternalOutput")

    with tile.TileContext(nc) as tc:
        with (
            tc.tile_pool(name="big", bufs=1) as bigp,
            tc.tile_pool(name="mb", bufs=2) as mbp,
            tc.tile_pool(name="psb", bufs=4) as psbp,
            tc.tile_pool(name="tmp", bufs=3) as tmpp,
            tc.tile_pool(name="ost", bufs=3) as ostp,
            tc.tile_pool(name="spps", bufs=3, space="PSUM") as spp,
            tc.tile_pool(name="outps", bufs=3, space="PSUM") as outp,
            tc.tile_pool(name="mps", bufs=1, space="PSUM") as mpp,
        ):
            qn_sb = bigp.tile([128, NW * 2 * WCOLS], f16)
            qt_sb = bigp.tile([64, NW * 2 * WCOLS], f16)
            cst_sb = bigp.tile([128, 512], f16)
            mask4 = cst_sb[:, 0:512]

            qn4 = qn.rearrange("p (w s q) -> p w s q", w=NW, s=2)
            qnsb4 = qn_sb.rearrange("p (w s q) -> p w s q", w=NW, s=2)
            qt4 = qt.rearrange("p (w q) -> p w q", w=NW)
            qtsb4 = qt_sb.rearrange("p (w q) -> p w q", w=NW)

            def dma_qn(w, clo, chi):
                a, b = 256 * clo, 256 * chi
                nc.sync.dma_start(qnsb4[:, w, :, a:b], qn4[:, w, :, a:b])

            def dma_qt(w, clo, chi):
                a, b = 512 * clo, 512 * chi
                nc.sync.dma_start(qtsb4[:, w, a:b], qt4[:, w, a:b])

            mreg = mpp.tile([64, 256], f32, name="mreg")

            # per-chunk records for the 2-chunk-lagged intra
            rec = {}

            def body(c):
                w, cl = c // CPW, c % CPW
                pw, k = cl // 2, cl % 2
                base = w * 2 * WCOLS

                def qr_sl(h):  # [128, 64] natural rope(Q) chunk
                    off = base + 256 * cl + 64 * h
                    return qn_sb[:, off:off + 64]

                def v_sl(h):   # [128, 64] V chunk
                    off = base + WCOLS + 256 * cl + 64 * h
                    return qn_sb[:, off:off + 64]

                def qrt_sl(h):  # [64, 128] transposed rope(Q) chunk
                    off = 2 * w * WCOLS + 512 * cl + 128 * h
                    return qt_sb[:, off:off + 128]

                # state: M_h += QR_c^T V_c   (PSUM accumulate across chunks)
                for h in range(HPC):
                    nc.tensor.matmul(
                        mreg[:, 64 * h:64 * h + 64],
                        qr_sl(h), v_sl(h),
                        start=(c == 0 and h == 0),
                        stop=(c == CH - 1 and h == HPC - 1),
                        skip_group_check=True,
                    )

                # M snapshot for inter of chunk c+1
                mb = None
                if c < CH - 1:
                    mb = mbp.tile([64, 256], f16, tag="mb")
                    if c % 2 == 0:
                        nc.vector.tensor_copy(mb[:], mreg[:])
                    else:
                        nc.scalar.copy(mb[:], mreg[:])

                # output PSUM tile per pair
                if k == 0:
                    op = outp.tile([128, 512], f32, tag="outp")
                else:
                    op = rec[c - 1]["op"]

                # S blocks (+ inter sharing the same stationary operand)
                sp = spp.tile([128, 512], f32, tag="sp")
                for h in range(HPC):
                    qrt_c = qrt_sl(h)
                    nc.tensor.matmul(
                        sp[:, 128 * h:128 * h + 128], qrt_c, qrt_c,
                        start=(h == 0), stop=(h == HPC - 1),
                    )
                    if c > 0:
                        # first write of this pair's outp zero region gets
                        # start=True (inter of even chunk; chunk 1 for pair 0)
                        nc.tensor.matmul(
                            op[:, 256 * k + 64 * h:256 * k + 64 * h + 64],
                            qrt_c, rec[c - 1]["mb"][:, 64 * h:64 * h + 64],
                            start=(h == 0 and (k == 0 or c == 1)),
                            stop=False,
                        )

                # P = S * strict-upper mask  (psum f32 -> sbuf fp16)
                psb = psbp.tile([128, 512], f16, tag="psb")
                r = c % 4
                if r in (1, 3):
                    nc.vector.tensor_mul(psb[:], sp[:], mask4)
                else:
                    tmp = tmpp.tile([128, 512], f16, tag="tmp")
                    nc.scalar.copy(tmp[:], sp[:])
                    if r == 0:
                        nc.vector.tensor_mul(psb[:], tmp[:], mask4)
                    else:
                        nc.gpsimd.tensor_mul(psb[:], tmp[:], mask4)

                # intra lagged by 2 chunks so the mask never stalls the PE
                if c > 1:
                    intra(c - 2)

                rec[c] = {"mb": mb, "psb": psb, "op": op,
                          "v": [v_sl(h) for h in range(HPC)]}
                rec.pop(c - 3, None)

            def intra(c):
                w, cl = c // CPW, c % CPW
                pw, k = cl // 2, cl % 2
                r = rec[c]
                for h in range(HPC):
                    nc.tensor.matmul(
                        r["op"][:, 256 * k + 64 * h:256 * k + 64 * h + 64],
                        r["psb"][:, 128 * h:128 * h + 128], r["v"][h],
                        start=False, stop=(k == 1 and h == HPC - 1),
                    )
                if k == 1:
                    # pair finished: fp16 staging copy + per-pair output DMA
                    ost = ostp.tile([128, 512], f16, tag="ost")
                    if pw % 2 == 0:
                        nc.scalar.copy(ost[:], r["op"][:])
                    else:
                        nc.vector.tensor_copy(ost[:], r["op"][:])
                    off = w * WCOLS + 512 * pw
                    nc.sync.dma_start(o[:, off:off + 512], ost[:])

            # ---- schedule ----
            # prologue: first chunks issued on separate queues so issue
            # overhead overlaps; compute starts after ~1 chunk of data
            def dma_qn_q(eng, w, clo, chi):
                a, b = 256 * clo, 256 * chi
                eng.dma_start(qnsb4[:, w, :, a:b], qn4[:, w, :, a:b])

            def dma_qt_q(eng, w, clo, chi):
                a, b = 512 * clo, 512 * chi
                eng.dma_start(qtsb4[:, w, a:b], qt4[:, w, a:b])

            dma_qn_q(nc.sync, 0, 0, 1)
            dma_qt_q(nc.scalar, 0, 0, 1)
            dma_qn_q(nc.sync, 0, 1, 2)
            dma_qt_q(nc.scalar, 0, 1, 2)
            nc.scalar.dma_start(cst_sb[:], cst[:])
            dma_qn_q(nc.sync, 0, 2, 4)
            dma_qt_q(nc.scalar, 0, 2, 4)
            dma_qn_q(nc.sync, 0, 4, CPW)
            dma_qt_q(nc.scalar, 0, 4, CPW)

            for c in range(CH):
                w, cl = c // CPW, c % CPW
                if w < NW - 1:
                    if cl == 0:
                        dma_qt(w + 1, 0, CPW)
                    elif cl == 1:
                        dma_qn(w + 1, 0, CPW)
                body(c)
            intra(CH - 2)
            intra(CH - 1)

    nc.compile()
    return nc


_CACHE = {}


def _get_program():
    if "nc" not in _CACHE:
        _CACHE["nc"] = build_program()
    return _CACHE["nc"]


def _tables():
    n = np.arange(N, dtype=np.float64)
    tq = np.floor(n / 2.0) * 2.0
    freqs = 1.0 / (THETA ** (tq / N)) / (2.0 * math.pi)
    t = np.arange(T, dtype=np.float64)[:, None]
    ang = ((t * freqs[None, :]) % 1.0) * (2.0 * math.pi)
    scale = float(N) ** -0.25
    cc = (np.cos(ang) * scale).astype(np.float32)
    ss = (np.sin(ang) * scale).astype(np.float32)
    ss[:, 0::2] *= -1.0
    return cc, ss


def make_inputs(Q, V):
    """Full inputs -> list of per-core {'qn','qt','cst'} fp16 host arrays."""
    Q = np.asarray(Q, dtype=np.float32).reshape(NCORES, HPC, T, N)
    V = np.asarray(V, dtype=np.float32).reshape(NCORES, HPC, T, N)
    cc, ss = _tables()
    sq = np.empty_like(Q)
    sq[..., 0::2] = Q[..., 1::2]
    sq[..., 1::2] = Q[..., 0::2]
    qr = (Q * cc + sq * ss).astype(np.float16)  # scaled rope(Q)
    v16 = V.astype(np.float16)

    # natural: [core, h, w, cw, p, n] -> [core, p, w, (cw h n)]
    def nat(x):
        x = x.reshape(NCORES, HPC, NW, CPW, 128, N)
        return np.transpose(x, (0, 4, 2, 3, 1, 5))  # core p w cw h n

    qn_h = np.stack([nat(qr), nat(v16)], axis=3)  # core p w s cw h n
    qn_h = np.ascontiguousarray(qn_h.reshape(NCORES, 128, NW * 2 * WCOLS))

    # transposed: [core, n, w, (cw h t)]
    qt_h = qr.reshape(NCORES, HPC, NW, CPW, 128, N)
    qt_h = np.transpose(qt_h, (0, 5, 2, 3, 1, 4))  # core n w cw h t
    qt_h = np.ascontiguousarray(qt_h.reshape(NCORES, 64, NW * 2 * WCOLS))

    mu = np.triu(np.ones((128, 128), dtype=np.float16), k=1)
    cst = np.ascontiguousarray(np.concatenate([mu] * 4, axis=1))  # [128, 512]
    return [{"qn": qn_h[i], "qt": qt_h[i], "cst": cst}
            for i in range(NCORES)]


def unpack_out(results):
    """list of per-core {'o': [128, NW*WCOLS] fp16} -> [B,H,T,N] f32."""
    o = np.stack([r["o"] for r in results], axis=0)
    o = o.reshape(NCORES, 128, NW, CPW, HPC, N)
    o = np.transpose(o, (0, 4, 2, 3, 1, 5))  # [8, HPC, NW, CPW, 128, N]
    return np.ascontiguousarray(
        o.reshape(B, H, T, N).astype(np.float32))


def kernel(Q, V):
    from concourse.bass_utils import run_bass_kernel_spmd

    nc = _get_program()
    in_maps = make_inputs(Q, V)
    res = run_bass_kernel_spmd(nc, in_maps, core_ids=list(range(NCORES)))
    return unpack_out(res.results)
